# revision 25
# baseline (speedup 1.0000x reference)
import sys

sys.path.insert(0, "/opt/trn_rl_repo")

import numpy as np

import concourse.bacc as bacc
import concourse.tile as tile
from concourse import mybir
from concourse.bass_utils import run_bass_kernel_spmd
from concourse.masks import make_identity

# Problem constants (hardcoded per contract): b=8 batches, one per core.
B = 8
N, P, H = 4096, 16, 128
HID, RD = 128, 64
Q, C = 128, 32  # n = q*C + c : partition q holds rows q*C .. q*C+C-1
G = 8           # DMA/convert groups of C//G chunks
CG = C // G
R = 4           # h-replication factor for packed broadcast APs
HB = H // R

F32 = mybir.dt.float32
F16 = mybir.dt.float16
ALU = mybir.AluOpType
ACT = mybir.ActivationFunctionType


def _build_nc(reps=1):
    nc = bacc.Bacc(None, target_bir_lowering=False)

    pf = nc.dram_tensor("pf", [N, H], F32, kind="ExternalInput")
    am = nc.dram_tensor("am", [N, P], F32, kind="ExternalInput")
    sq = nc.dram_tensor("sq", [P, H], F32, kind="ExternalInput")
    w1 = nc.dram_tensor("w1", [4 * H, HID], F32, kind="ExternalInput")
    b1 = nc.dram_tensor("b1", [1, HID], F32, kind="ExternalInput")
    w2 = nc.dram_tensor("w2", [HID, RD], F32, kind="ExternalInput")
    b2 = nc.dram_tensor("b2", [1, RD], F32, kind="ExternalInput")
    out = nc.dram_tensor("out", [P, RD], F32, kind="ExternalOutput")

    with tile.TileContext(nc) as tc:
        with (
            tc.tile_pool(name="big", bufs=1) as big,
            tc.tile_pool(name="small", bufs=1) as small,
            tc.tile_pool(name="pacc", bufs=1, space="PSUM") as pacc,
            tc.tile_pool(name="ptr", bufs=1, space="PSUM") as ptr,
            tc.tile_pool(name="pseq", bufs=2, space="PSUM") as pseq,
        ):
            sq_sb = small.tile([P, H], F32)
            nc.sync.dma_start(out=sq_sb[:], in_=sq[:])
            w1_sb = small.tile([Q, 4, HID], F32)
            nc.sync.dma_start(out=w1_sb[:], in_=w1[:].rearrange("(i k) m -> k i m", i=4))
            b1_sb = small.tile([1, HID], F32)
            nc.sync.dma_start(out=b1_sb[:], in_=b1[:])
            w2_sb = small.tile([HID, RD], F32)
            nc.sync.dma_start(out=w2_sb[:], in_=w2[:])
            b2_sb = small.tile([1, RD], F32)
            nc.sync.dma_start(out=b2_sb[:], in_=b2[:])

            ident16 = small.tile([Q, Q], F16)
            make_identity(nc, ident16[:])
            ident32 = small.tile([P, P], F32)
            make_identity(nc, ident32[:])
            ones16 = small.tile([Q, 1], F16)
            nc.vector.memset(ones16[:], 1.0)
            ones_row = small.tile([1, P], F32)
            nc.vector.memset(ones_row[:], 1.0)
            ones_col = small.tile([1, Q], F32)
            nc.vector.memset(ones_col[:], 1.0)

            for _rep in range(reps):
                _build_body(
                    nc, big, small, pacc, ptr, pseq,
                    pf, am, out,
                    sq_sb, w1_sb, b1_sb, w2_sb, b2_sb,
                    ident16, ident32, ones16, ones_row, ones_col,
                )

    nc.finalize()
    return nc


def _build_body(
    nc, big, small, pacc, ptr, pseq,
    pf, am, out,
    sq_sb, w1_sb, b1_sb, w2_sb, b2_sb,
    ident16, ident32, ones16, ones_row, ones_col,
):
    pf32 = big.tile([Q, C, H], F32, tag="pf32")
    pf16 = big.tile([Q, C, H], F16, tag="pf16")
    pf2 = big.tile([Q, C, H], F16, tag="pf2")
    a32 = big.tile([Q, C, P], F32, tag="a32")
    a16 = big.tile([Q, C, P], F16, tag="a16")
    arep = big.tile([Q, C, P, R], F16, tag="arep")
    prodg = big.tile([Q, CG, P, H], F16, tag="prodg")
    acc = big.tile([Q, P, H], F16, tag="acc")

    pf_r = pf[:].rearrange("(q c) h -> q c h", q=Q)
    am_r = am[:].rearrange("(q c) p -> q c p", q=Q)
    for g in range(G):
        cs = slice(g * CG, (g + 1) * CG)
        nc.sync.dma_start(out=pf32[:, cs, :], in_=pf_r[:, cs, :])
        nc.sync.dma_start(out=a32[:, cs, :], in_=am_r[:, cs, :])

    # f32 -> f16 conversions + squares + A replication on ScalarE, per group
    for g in range(G):
        cs = slice(g * CG, (g + 1) * CG)
        nc.scalar.activation(out=pf16[:, cs, :], in_=pf32[:, cs, :], func=ACT.Copy)
        nc.scalar.activation(out=pf2[:, cs, :], in_=pf32[:, cs, :], func=ACT.Square)
        nc.scalar.activation(out=a16[:, cs, :], in_=a32[:, cs, :], func=ACT.Copy)
        nc.scalar.activation(
            out=arep[:, cs, :, :],
            in_=a32[:, cs, :, None].broadcast_to([Q, CG, P, R]),
            func=ACT.Copy,
        )

    # max path v2: per chunk one fat 2x TT mult (prod[q,p,h] = A[q,c,p]*PF[q,c,h]
    # via r-packed broadcast APs), then an in-place TT-max tree per group.
    for g in range(G):
        cs = slice(g * CG, (g + 1) * CG)
        in0 = (
            pf16[:, cs, :]
            .rearrange("q c (hb r) -> q c hb r", r=R)[:, :, None, :, :]
            .broadcast_to([Q, CG, P, HB, R])
        )
        in1 = arep[:, cs, :, None, :].broadcast_to([Q, CG, P, HB, R])
        outv = prodg[:].rearrange("q i p (hb r) -> q i p hb r", r=R)
        nc.vector.tensor_tensor(out=outv, in0=in0, in1=in1, op=ALU.mult)
        nc.vector.tensor_tensor(
            out=prodg[:, 0:2], in0=prodg[:, 0:2], in1=prodg[:, 2:4], op=ALU.max
        )
        if g == 0:
            nc.vector.tensor_tensor(
                out=acc[:], in0=prodg[:, 0], in1=prodg[:, 1], op=ALU.max
            )
        else:
            nc.vector.tensor_tensor(
                out=acc[:], in0=acc[:], in1=prodg[:, 0], op=ALU.max
            )
            nc.vector.tensor_tensor(
                out=acc[:], in0=acc[:], in1=prodg[:, 1], op=ALU.max
            )

    # PE accumulations: pooledT[h,p] = sum_n PF*A ; sqsumT[h,p] = sum_n PF^2*A ; mass[1,p]
    pooled_ps = pacc.tile([H, P], F32, tag="pooled_ps")
    sqsum_ps = pacc.tile([H, P], F32, tag="sqsum_ps")
    mass_ps = pacc.tile([1, P], F32, tag="mass_ps")
    for c in range(C):
        nc.tensor.matmul(pooled_ps[:], pf16[:, c, :], a16[:, c, :],
                         start=(c == 0), stop=(c == C - 1))
    for c in range(C):
        nc.tensor.matmul(sqsum_ps[:], pf2[:, c, :], a16[:, c, :],
                         start=(c == 0), stop=(c == C - 1))
    for c in range(C):
        nc.tensor.matmul(mass_ps[:], ones16[:], a16[:, c, :],
                         start=(c == 0), stop=(c == C - 1))

    # partition max: PE-transpose the 16 [q,h] planes into one PSUM tile,
    # then a single fat free-axis reduce
    psT = ptr.tile([Q, P, Q], F16, tag="psT")
    for p in range(P):
        nc.tensor.transpose(psT[:, p, :], acc[:, p, :], ident16[:])
    maxT = small.tile([Q, P], F32, tag="maxT")
    nc.vector.tensor_reduce(
        out=maxT[:], in_=psT[:], axis=mybir.AxisListType.X, op=ALU.max
    )

    # stats: pooledT = pooled/mass ; varT = sqsum/mass - pooledT^2
    recip = small.tile([1, P], F32, tag="recip")
    nc.vector.reciprocal(recip[:], mass_ps[:])
    recipb_ps = pseq.tile([Q, P], F32, tag="seq")
    nc.tensor.matmul(recipb_ps[:], ones_col[:], recip[:])
    recipb = small.tile([Q, P], F32, tag="recipb")
    nc.vector.tensor_copy(recipb[:], recipb_ps[:])

    pooledT = small.tile([Q, P], F32, tag="pooledT")
    nc.vector.tensor_mul(pooledT[:], pooled_ps[:], recipb[:])
    ex2T = small.tile([Q, P], F32, tag="ex2T")
    nc.vector.tensor_mul(ex2T[:], sqsum_ps[:], recipb[:])
    psq = small.tile([Q, P], F32, tag="psq")
    nc.vector.tensor_mul(psq[:], pooledT[:], pooledT[:])
    varT = small.tile([Q, P], F32, tag="varT")
    nc.vector.tensor_sub(varT[:], ex2T[:], psq[:])

    # sqT[h,p] via PE transpose of sq_sb [16,128]
    sqT_ps = pseq.tile([Q, P], F32, tag="seq")
    nc.tensor.transpose(sqT_ps[:], sq_sb[:], ident32[:])
    sqT = small.tile([Q, P], F32, tag="sqT")
    nc.vector.tensor_copy(sqT[:], sqT_ps[:])

    # MLP layer 1: hdn[p,hid] = relu([sq|pooled|max|var] @ W1 + b1)
    hdn_ps = pseq.tile([P, HID], F32, tag="seq")
    nc.tensor.matmul(hdn_ps[:], sqT[:], w1_sb[:, 0, :], start=True, stop=False)
    nc.tensor.matmul(hdn_ps[:], pooledT[:], w1_sb[:, 1, :], start=False, stop=False)
    nc.tensor.matmul(hdn_ps[:], maxT[:], w1_sb[:, 2, :], start=False, stop=False)
    nc.tensor.matmul(hdn_ps[:], varT[:], w1_sb[:, 3, :], start=False, stop=False)
    nc.tensor.matmul(hdn_ps[:], ones_row[:], b1_sb[:], start=False, stop=True)
    hdn = small.tile([P, HID], F32, tag="hdn")
    nc.scalar.activation(out=hdn[:], in_=hdn_ps[:], func=ACT.Relu)

    # MLP layer 2: out[p,rd] = hdn @ W2 + b2
    hdnT_ps = pseq.tile([HID, P], F32, tag="seq")
    nc.tensor.transpose(hdnT_ps[:], hdn[:], ident32[:])
    hdnT = small.tile([HID, P], F32, tag="hdnT")
    nc.vector.tensor_copy(hdnT[:], hdnT_ps[:])

    out_ps = pseq.tile([P, RD], F32, tag="seq")
    nc.tensor.matmul(out_ps[:], hdnT[:], w2_sb[:], start=True, stop=False)
    nc.tensor.matmul(out_ps[:], ones_row[:], b2_sb[:], start=False, stop=True)
    out_sb = small.tile([P, RD], F32, tag="out_sb")
    nc.vector.tensor_copy(out_sb[:], out_ps[:])
    nc.sync.dma_start(out=out[:], in_=out_sb[:])


_NC = None
TRACE = False
LAST_RESULT = None


def _get_nc():
    global _NC
    if _NC is None:
        _NC = _build_nc()
    return _NC


def kernel(sq_features, point_features, assign_matrix, W1, b1, W2, b2):
    sq_features = np.asarray(sq_features, np.float32)
    point_features = np.asarray(point_features, np.float32)
    assign_matrix = np.asarray(assign_matrix, np.float32)
    W1 = np.ascontiguousarray(np.asarray(W1, np.float32))
    b1 = np.ascontiguousarray(np.asarray(b1, np.float32).reshape(1, HID))
    W2 = np.ascontiguousarray(np.asarray(W2, np.float32))
    b2 = np.ascontiguousarray(np.asarray(b2, np.float32).reshape(1, RD))

    nc = _get_nc()
    in_maps = []
    for i in range(B):
        in_maps.append(
            {
                "pf": np.ascontiguousarray(point_features[i]),
                "am": np.ascontiguousarray(assign_matrix[i]),
                "sq": np.ascontiguousarray(sq_features[i]),
                "w1": W1,
                "b1": b1,
                "w2": W2,
                "b2": b2,
            }
        )
    res = run_bass_kernel_spmd(nc, in_maps, core_ids=list(range(B)), trace=TRACE)
    global LAST_RESULT
    LAST_RESULT = res
    return np.stack([np.asarray(res.results[i]["out"]) for i in range(B)]).astype(
        np.float32
    )


# revision 26
# speedup vs baseline: 1.0905x; 1.0905x over previous
import sys

sys.path.insert(0, "/opt/trn_rl_repo")

import numpy as np

import concourse.bacc as bacc
import concourse.tile as tile
from concourse import mybir
from concourse.bass_utils import run_bass_kernel_spmd
from concourse.masks import make_identity

# Problem constants (hardcoded per contract): b=8 batches, one per core.
B = 8
N, P, H = 4096, 16, 128
HID, RD = 128, 64
Q, C = 128, 32  # n = q*C + c : partition q holds rows q*C .. q*C+C-1
G = 4           # DMA/convert groups of C//G chunks
CG = C // G
R = 4           # h-replication factor for packed broadcast APs
HB = H // R

F32 = mybir.dt.float32
F16 = mybir.dt.float16
ALU = mybir.AluOpType
ACT = mybir.ActivationFunctionType


def _build_nc(reps=1):
    nc = bacc.Bacc(None, target_bir_lowering=False)

    pf = nc.dram_tensor("pf", [N, H], F32, kind="ExternalInput")
    am = nc.dram_tensor("am", [N, P], F32, kind="ExternalInput")
    sq = nc.dram_tensor("sq", [P, H], F32, kind="ExternalInput")
    w1 = nc.dram_tensor("w1", [4 * H, HID], F32, kind="ExternalInput")
    b1 = nc.dram_tensor("b1", [1, HID], F32, kind="ExternalInput")
    w2 = nc.dram_tensor("w2", [HID, RD], F32, kind="ExternalInput")
    b2 = nc.dram_tensor("b2", [1, RD], F32, kind="ExternalInput")
    out = nc.dram_tensor("out", [P, RD], F32, kind="ExternalOutput")

    with tile.TileContext(nc) as tc:
        with (
            tc.tile_pool(name="big", bufs=1) as big,
            tc.tile_pool(name="small", bufs=1) as small,
            tc.tile_pool(name="pacc", bufs=1, space="PSUM") as pacc,
            tc.tile_pool(name="ptr", bufs=1, space="PSUM") as ptr,
            tc.tile_pool(name="pseq", bufs=2, space="PSUM") as pseq,
        ):
            sq_sb = small.tile([P, H], F32)
            nc.sync.dma_start(out=sq_sb[:], in_=sq[:])
            w1_sb = small.tile([Q, 4, HID], F32)
            nc.sync.dma_start(out=w1_sb[:], in_=w1[:].rearrange("(i k) m -> k i m", i=4))
            b1_sb = small.tile([1, HID], F32)
            nc.sync.dma_start(out=b1_sb[:], in_=b1[:])
            w2_sb = small.tile([HID, RD], F32)
            nc.sync.dma_start(out=w2_sb[:], in_=w2[:])
            b2_sb = small.tile([1, RD], F32)
            nc.sync.dma_start(out=b2_sb[:], in_=b2[:])

            ident16 = small.tile([Q, Q], F16)
            make_identity(nc, ident16[:])
            ident32 = small.tile([P, P], F32)
            make_identity(nc, ident32[:])
            ones16 = small.tile([Q, 1], F16)
            nc.vector.memset(ones16[:], 1.0)
            ones_row = small.tile([1, P], F32)
            nc.vector.memset(ones_row[:], 1.0)
            ones_col = small.tile([1, Q], F32)
            nc.vector.memset(ones_col[:], 1.0)

            for _rep in range(reps):
                _build_body(
                    nc, big, small, pacc, ptr, pseq,
                    pf, am, out,
                    sq_sb, w1_sb, b1_sb, w2_sb, b2_sb,
                    ident16, ident32, ones16, ones_row, ones_col,
                )

    nc.finalize()
    return nc


def _build_body(
    nc, big, small, pacc, ptr, pseq,
    pf, am, out,
    sq_sb, w1_sb, b1_sb, w2_sb, b2_sb,
    ident16, ident32, ones16, ones_row, ones_col,
):
    pf32 = big.tile([Q, C, H], F32, tag="pf32")
    pf16 = big.tile([Q, C, H], F16, tag="pf16")
    pf2 = big.tile([Q, C, H], F16, tag="pf2")
    a32 = big.tile([Q, C, P], F32, tag="a32")
    a16 = big.tile([Q, C, P], F16, tag="a16")
    arep = big.tile([Q, C, P, R], F16, tag="arep")
    prodg = big.tile([Q, CG, P, H], F16, tag="prodg")
    acc = big.tile([Q, P, H], F16, tag="acc")

    pf_r = pf[:].rearrange("(q c) h -> q c h", q=Q)
    am_r = am[:].rearrange("(q c) p -> q c p", q=Q)
    for g in range(G):
        cs = slice(g * CG, (g + 1) * CG)
        nc.sync.dma_start(out=pf32[:, cs, :], in_=pf_r[:, cs, :])
        nc.sync.dma_start(out=a32[:, cs, :], in_=am_r[:, cs, :])

    # f32 -> f16 conversions + squares + A replication on ScalarE, per group
    for g in range(G):
        cs = slice(g * CG, (g + 1) * CG)
        nc.scalar.activation(out=pf16[:, cs, :], in_=pf32[:, cs, :], func=ACT.Copy)
        nc.scalar.activation(out=pf2[:, cs, :], in_=pf32[:, cs, :], func=ACT.Square)
        nc.scalar.activation(out=a16[:, cs, :], in_=a32[:, cs, :], func=ACT.Copy)
        nc.scalar.activation(
            out=arep[:, cs, :, :],
            in_=a32[:, cs, :, None].broadcast_to([Q, CG, P, R]),
            func=ACT.Copy,
        )

    # max path v2: per chunk one fat 2x TT mult (prod[q,p,h] = A[q,c,p]*PF[q,c,h]
    # via r-packed broadcast APs), then an in-place TT-max tree per group.
    for g in range(G):
        cs = slice(g * CG, (g + 1) * CG)
        in0 = (
            pf16[:, cs, :]
            .rearrange("q c (hb r) -> q c hb r", r=R)[:, :, None, :, :]
            .broadcast_to([Q, CG, P, HB, R])
        )
        in1 = arep[:, cs, :, None, :].broadcast_to([Q, CG, P, HB, R])
        outv = prodg[:].rearrange("q i p (hb r) -> q i p hb r", r=R)
        nc.vector.tensor_tensor(out=outv, in0=in0, in1=in1, op=ALU.mult)
        nc.vector.tensor_tensor(
            out=prodg[:, 0:4], in0=prodg[:, 0:4], in1=prodg[:, 4:8], op=ALU.max
        )
        nc.vector.tensor_tensor(
            out=prodg[:, 0:2], in0=prodg[:, 0:2], in1=prodg[:, 2:4], op=ALU.max
        )
        if g == 0:
            nc.vector.tensor_tensor(
                out=acc[:], in0=prodg[:, 0], in1=prodg[:, 1], op=ALU.max
            )
        else:
            nc.vector.tensor_tensor(
                out=acc[:], in0=acc[:], in1=prodg[:, 0], op=ALU.max
            )
            nc.vector.tensor_tensor(
                out=acc[:], in0=acc[:], in1=prodg[:, 1], op=ALU.max
            )

    # PE accumulations: pooledT[h,p] = sum_n PF*A ; sqsumT[h,p] = sum_n PF^2*A ; mass[1,p]
    pooled_ps = pacc.tile([H, P], F32, tag="pooled_ps")
    sqsum_ps = pacc.tile([H, P], F32, tag="sqsum_ps")
    mass_ps = pacc.tile([1, P], F32, tag="mass_ps")
    for c in range(C):
        nc.tensor.matmul(pooled_ps[:], pf16[:, c, :], a16[:, c, :],
                         start=(c == 0), stop=(c == C - 1))
    for c in range(C):
        nc.tensor.matmul(sqsum_ps[:], pf2[:, c, :], a16[:, c, :],
                         start=(c == 0), stop=(c == C - 1))
    for c in range(C):
        nc.tensor.matmul(mass_ps[:], ones16[:], a16[:, c, :],
                         start=(c == 0), stop=(c == C - 1))

    # partition max: PE-transpose the 16 [q,h] planes into one PSUM tile,
    # then a single fat free-axis reduce
    psT = ptr.tile([Q, P, Q], F16, tag="psT")
    for p in range(P):
        nc.tensor.transpose(psT[:, p, :], acc[:, p, :], ident16[:])
    maxT = small.tile([Q, P], F32, tag="maxT")
    nc.vector.tensor_reduce(
        out=maxT[:], in_=psT[:], axis=mybir.AxisListType.X, op=ALU.max
    )

    # stats: pooledT = pooled/mass ; varT = sqsum/mass - pooledT^2
    recip = small.tile([1, P], F32, tag="recip")
    nc.vector.reciprocal(recip[:], mass_ps[:])
    recipb_ps = pseq.tile([Q, P], F32, tag="seq")
    nc.tensor.matmul(recipb_ps[:], ones_col[:], recip[:])
    recipb = small.tile([Q, P], F32, tag="recipb")
    nc.vector.tensor_copy(recipb[:], recipb_ps[:])

    pooledT = small.tile([Q, P], F32, tag="pooledT")
    nc.vector.tensor_mul(pooledT[:], pooled_ps[:], recipb[:])
    ex2T = small.tile([Q, P], F32, tag="ex2T")
    nc.vector.tensor_mul(ex2T[:], sqsum_ps[:], recipb[:])
    psq = small.tile([Q, P], F32, tag="psq")
    nc.vector.tensor_mul(psq[:], pooledT[:], pooledT[:])
    varT = small.tile([Q, P], F32, tag="varT")
    nc.vector.tensor_sub(varT[:], ex2T[:], psq[:])

    # sqT[h,p] via PE transpose of sq_sb [16,128]
    sqT_ps = pseq.tile([Q, P], F32, tag="seq")
    nc.tensor.transpose(sqT_ps[:], sq_sb[:], ident32[:])
    sqT = small.tile([Q, P], F32, tag="sqT")
    nc.vector.tensor_copy(sqT[:], sqT_ps[:])

    # MLP layer 1: hdn[p,hid] = relu([sq|pooled|max|var] @ W1 + b1)
    hdn_ps = pseq.tile([P, HID], F32, tag="seq")
    nc.tensor.matmul(hdn_ps[:], sqT[:], w1_sb[:, 0, :], start=True, stop=False)
    nc.tensor.matmul(hdn_ps[:], pooledT[:], w1_sb[:, 1, :], start=False, stop=False)
    nc.tensor.matmul(hdn_ps[:], maxT[:], w1_sb[:, 2, :], start=False, stop=False)
    nc.tensor.matmul(hdn_ps[:], varT[:], w1_sb[:, 3, :], start=False, stop=False)
    nc.tensor.matmul(hdn_ps[:], ones_row[:], b1_sb[:], start=False, stop=True)
    hdn = small.tile([P, HID], F32, tag="hdn")
    nc.scalar.activation(out=hdn[:], in_=hdn_ps[:], func=ACT.Relu)

    # MLP layer 2: out[p,rd] = hdn @ W2 + b2
    hdnT_ps = pseq.tile([HID, P], F32, tag="seq")
    nc.tensor.transpose(hdnT_ps[:], hdn[:], ident32[:])
    hdnT = small.tile([HID, P], F32, tag="hdnT")
    nc.vector.tensor_copy(hdnT[:], hdnT_ps[:])

    out_ps = pseq.tile([P, RD], F32, tag="seq")
    nc.tensor.matmul(out_ps[:], hdnT[:], w2_sb[:], start=True, stop=False)
    nc.tensor.matmul(out_ps[:], ones_row[:], b2_sb[:], start=False, stop=True)
    out_sb = small.tile([P, RD], F32, tag="out_sb")
    nc.vector.tensor_copy(out_sb[:], out_ps[:])
    nc.sync.dma_start(out=out[:], in_=out_sb[:])


_NC = None
TRACE = False
LAST_RESULT = None


def _get_nc():
    global _NC
    if _NC is None:
        _NC = _build_nc()
    return _NC


def kernel(sq_features, point_features, assign_matrix, W1, b1, W2, b2):
    sq_features = np.asarray(sq_features, np.float32)
    point_features = np.asarray(point_features, np.float32)
    assign_matrix = np.asarray(assign_matrix, np.float32)
    W1 = np.ascontiguousarray(np.asarray(W1, np.float32))
    b1 = np.ascontiguousarray(np.asarray(b1, np.float32).reshape(1, HID))
    W2 = np.ascontiguousarray(np.asarray(W2, np.float32))
    b2 = np.ascontiguousarray(np.asarray(b2, np.float32).reshape(1, RD))

    nc = _get_nc()
    in_maps = []
    for i in range(B):
        in_maps.append(
            {
                "pf": np.ascontiguousarray(point_features[i]),
                "am": np.ascontiguousarray(assign_matrix[i]),
                "sq": np.ascontiguousarray(sq_features[i]),
                "w1": W1,
                "b1": b1,
                "w2": W2,
                "b2": b2,
            }
        )
    res = run_bass_kernel_spmd(nc, in_maps, core_ids=list(range(B)), trace=TRACE)
    global LAST_RESULT
    LAST_RESULT = res
    return np.stack([np.asarray(res.results[i]["out"]) for i in range(B)]).astype(
        np.float32
    )
